# revision 6
# baseline (speedup 1.0000x reference)
"""Trainium2 Bass kernel for nn_MILPFAttnTrexModel (segment_reduce).

Contract: kernel(**inputs) takes the FULL unsharded inputs (numpy arrays, keys
as in reference.setup_inputs()) and returns the FULL [G, NC] float32 output.

Strategy (8 NeuronCores, SPMD — one program, per-core data):
  - Host buckets rows by group; 8 groups per core, each group's tile-instance
    rows padded to a uniform block of TB columns (TB multiple of 384), whole-
    instance rows padded to WB columns. Inputs are shipped pre-transposed
    (feature-major, [1024, cols]) so the K (contraction) dim lands on SBUF
    partitions.
  - Device, per group: 2-layer MLP (feature-major) -> scores via folded
    Wk@latent.T/sqrt(LC) -> segment softmax (free-dim reduce_max + Exp with
    accumulated denominator) -> v row-major + PE-transposed ex -> per-group
    [L, LC] weighted sum accumulated on PSUM.
    Pad columns are killed exactly by an extra K-row in the scores matmul
    contributing -1e30 * pad_flag (host data), so the one SPMD program is
    valid for every core's bucket sizes.
  - Whole-image branch: same MLP shape with Wg*, per-group free-dim
    reduce_max. Pad columns are forced to 0 pre-relu by the same -1e30 trick;
    real columns are relu outputs >= 0 so the max is unaffected.
  - Host: bv add, whole_agg/out_group assembly, final fused @ Wout + bout.
"""

import math
import os
import numpy as np

import concourse.bacc as bacc
import concourse.tile as tile
from concourse import mybir
from concourse.bass_utils import run_bass_kernel_spmd
from concourse.masks import make_identity

# Set by the most recent kernel() call when KERNEL_TRACE=1 (dev-only).
last_exec_time_ns = None
last_mean_exec_time_ns = None


def _install_ntff_shim():
    """Register the axon NTFF profile hook if the image's antenv lacks it."""
    import sys, types
    try:
        import antenv.axon_hooks  # noqa: F401
        return
    except ImportError:
        pass
    m = types.ModuleType("antenv.axon_hooks")
    m._hook = None
    m.set_axon_ntff_profile_hook = lambda h: setattr(m, "_hook", h)
    m.get_axon_ntff_profile_hook = lambda: m._hook
    sys.modules["antenv.axon_hooks"] = m
    import antenv
    antenv.axon_hooks = m
    from trn_agent_boot.trn_boot import _ntff_profile_via_ctypes
    m.set_axon_ntff_profile_hook(
        _ntff_profile_via_ctypes("/opt/axon/libaxon_pjrt.so"))

F32 = mybir.dt.float32
F32R = mybir.dt.float32r
AX = mybir.AxisListType
ALU = mybir.AluOpType
ACTF = mybir.ActivationFunctionType

N_CORES = 8
G = 64
GPC = G // N_CORES          # groups per core
IN = 1024
GL = 512
LC = 256
L = 8
NCLS = 2
NEGBIG = -1.0e30

_prog_cache = {}


def _ceil_to(x, m):
    return ((x + m - 1) // m) * m


def _build_program(TB, WB, tile_aug, whole_aug):
    """Build the SPMD Tile program for block sizes (TB, WB)."""
    T = GPC * TB
    Wt = GPC * WB
    NCH = TB // 384          # scores/psum N-chunks per group
    NSZ = 384
    RC = TB // 128           # 128-row chunks per group
    # whole-branch N chunks (<=512, may be ragged)
    wchunks = []
    off = 0
    while off < Wt:
        sz = min(512, Wt - off)
        wchunks.append((off, sz))
        off += sz

    nc = bacc.Bacc("TRN2", target_bir_lowering=False, debug=False,
                   num_devices=N_CORES)

    xtt = nc.dram_tensor("xtt", [IN, T], F32, kind="ExternalInput")
    xwt = nc.dram_tensor("xwt", [IN, Wt], F32, kind="ExternalInput")
    padf = nc.dram_tensor("padf", [1, T], F32, kind="ExternalInput")
    padfw = nc.dram_tensor("padfw", [1, Wt], F32, kind="ExternalInput")
    wl0 = nc.dram_tensor("wl0", [IN, GL], F32, kind="ExternalInput")
    wl1 = nc.dram_tensor("wl1", [GL, LC], F32, kind="ExternalInput")
    wv = nc.dram_tensor("wv", [LC, LC], F32, kind="ExternalInput")
    wkl = nc.dram_tensor("wkl", [LC, L], F32, kind="ExternalInput")
    wg0 = nc.dram_tensor("wg0", [IN, 2 * GL], F32, kind="ExternalInput")
    wg1 = nc.dram_tensor("wg1", [2 * GL, GL], F32, kind="ExternalInput")
    bl0t = nc.dram_tensor("bl0t", [128, GL // 128], F32, kind="ExternalInput")
    bl1t = nc.dram_tensor("bl1t", [128, LC // 128], F32, kind="ExternalInput")
    bg0t = nc.dram_tensor("bg0t", [128, 2 * GL // 128], F32, kind="ExternalInput")
    bg1t = nc.dram_tensor("bg1t", [128, GL // 128], F32, kind="ExternalInput")
    negbig_in = nc.dram_tensor("negbig", [1, 128], F32, kind="ExternalInput")
    out_og = nc.dram_tensor("out_og", [L, GPC, LC], F32, kind="ExternalOutput")
    out_w = nc.dram_tensor("out_w", [128, GL // 128, GPC], F32,
                           kind="ExternalOutput")

    tick = [0]

    def evac(out_ap, in_ap, bias_ap=None):
        """PSUM -> SBUF eviction, optionally fused bias-add + relu.
        Alternates DVE / ACT to balance engine load."""
        use_dve = (tick[0] % 2 == 0)
        tick[0] += 1
        if bias_ap is None:
            if use_dve:
                nc.vector.tensor_copy(out_ap, in_ap)
            else:
                nc.scalar.copy(out_ap, in_ap)
        else:
            if use_dve:
                nc.vector.tensor_scalar(out_ap, in_ap, bias_ap, 0.0,
                                        op0=ALU.add, op1=ALU.max)
            else:
                nc.scalar.activation(out_ap, in_ap, ACTF.Relu, bias=bias_ap)

    with tile.TileContext(nc) as tc:
        with tc.tile_pool(name="weights", bufs=1) as wpool:
            wl0_sb = wpool.tile([128, IN // 128, GL], F32R)
            nc.scalar.dma_start(out=wl0_sb,
                              in_=wl0.ap().bitcast(F32R).rearrange(
                                  "(kt p) m -> p kt m", p=128))
            wl1_sb = wpool.tile([128, GL // 128, LC], F32R)
            nc.scalar.dma_start(out=wl1_sb,
                              in_=wl1.ap().bitcast(F32R).rearrange(
                                  "(kt p) m -> p kt m", p=128))
            wv_sb = wpool.tile([128, LC // 128, LC], F32R)
            nc.scalar.dma_start(out=wv_sb,
                              in_=wv.ap().bitcast(F32R).rearrange(
                                  "(kt p) m -> p kt m", p=128))
            wkl_sb = wpool.tile([128, LC // 128, L], F32R)
            nc.scalar.dma_start(out=wkl_sb,
                              in_=wkl.ap().bitcast(F32R).rearrange(
                                  "(kt p) m -> p kt m", p=128))
            bl0_sb = wpool.tile([128, GL // 128], F32)
            nc.scalar.dma_start(out=bl0_sb, in_=bl0t.ap())
            bl1_sb = wpool.tile([128, LC // 128], F32)
            nc.scalar.dma_start(out=bl1_sb, in_=bl1t.ap())
            ident_sb = wpool.tile([128, 128], F32)
            make_identity(nc, ident_sb)
            negbig_sb = wpool.tile([1, 128], F32R)
            nc.scalar.dma_start(out=negbig_sb, in_=negbig_in.ap().bitcast(F32R))

            # ---------------- tile-instance branch, per group ----------------
            with (
                tc.tile_pool(name="xt", bufs=2) as xtpool,
                tc.tile_pool(name="h1", bufs=2) as h1pool,
                tc.tile_pool(name="xt2", bufs=1) as xt2pool,
                tc.tile_pool(name="scex", bufs=1) as scpool,
                tc.tile_pool(name="vrm", bufs=2) as vpool,
                tc.tile_pool(name="ext", bufs=2) as extpool,
                tc.tile_pool(name="small", bufs=2) as smpool,
                tc.tile_pool(name="ogall", bufs=1) as ogpool,
                tc.tile_pool(name="ph1", bufs=2, space="PSUM") as ph1,
                tc.tile_pool(name="psc", bufs=2, space="PSUM") as psc,
                tc.tile_pool(name="pv", bufs=2, space="PSUM") as pv,
                tc.tile_pool(name="pt", bufs=1, space="PSUM") as pt,
                tc.tile_pool(name="pog", bufs=1, space="PSUM") as pog,
            ):
                og_sb = ogpool.tile([L, GPC, LC], F32)
                xtt_r = xtt.ap().bitcast(F32R).rearrange("(kt p) t -> p kt t", p=128)

                for j in range(GPC):
                    c0 = j * TB
                    xt_sb = xtpool.tile([128, IN // 128, TB], F32R)
                    nc.sync.dma_start(out=xt_sb, in_=xtt_r[:, :, c0:c0 + TB])
                    pf_sb = smpool.tile([1, TB], F32R, tag="pf")
                    nc.scalar.dma_start(out=pf_sb,
                                      in_=padf.ap().bitcast(F32R)[0:1, c0:c0 + TB])

                    # L1: h1 = relu(Wl0.T @ xt + bl0)   [512, TB]
                    h1_sb = h1pool.tile([128, GL // 128, TB], F32R)
                    for mc in range(GL // 128):
                        for s in range(NCH):
                            n0 = s * NSZ
                            ps = ph1.tile([128, NSZ], F32, tag="ps")
                            for kt in range(IN // 128):
                                nc.tensor.matmul(
                                    ps, wl0_sb[:, kt, mc * 128:(mc + 1) * 128],
                                    xt_sb[:, kt, n0:n0 + NSZ],
                                    start=(kt == 0), stop=(kt == IN // 128 - 1))
                            evac(h1_sb[:, mc, n0:n0 + NSZ], ps,
                                 bl0_sb[:, mc:mc + 1])

                    # L2: xt2 = relu(Wl1.T @ h1 + bl1)  [256, TB]
                    xt2_sb = xt2pool.tile([128, LC // 128, TB], F32R)
                    for mc in range(LC // 128):
                        for s in range(NCH):
                            n0 = s * NSZ
                            ps = ph1.tile([128, NSZ], F32, tag="ps")
                            for kt in range(GL // 128):
                                nc.tensor.matmul(
                                    ps, wl1_sb[:, kt, mc * 128:(mc + 1) * 128],
                                    h1_sb[:, kt, n0:n0 + NSZ],
                                    start=(kt == 0), stop=(kt == GL // 128 - 1))
                            evac(xt2_sb[:, mc, n0:n0 + NSZ], ps,
                                 bl1_sb[:, mc:mc + 1])

                    # scores [L, TB] = WkLat.T @ xt2  (+ NEGBIG * pad_flag)
                    sc_sb = scpool.tile([L, TB], F32, tag="sc")
                    for s in range(NCH):
                        n0 = s * NSZ
                        ps = psc.tile([L, NSZ], F32, tag="psc")
                        for kt in range(LC // 128):
                            nc.tensor.matmul(
                                ps, wkl_sb[:, kt, :], xt2_sb[:, kt, n0:n0 + NSZ],
                                start=(kt == 0), stop=(not tile_aug and
                                                       kt == LC // 128 - 1))
                        if tile_aug:
                            nc.tensor.matmul(ps, negbig_sb[0:1, 0:L],
                                             pf_sb[0:1, n0:n0 + NSZ],
                                             start=False, stop=True)
                        evac(sc_sb[:, n0:n0 + NSZ], ps)

                    # segment softmax pieces (rows of this group only)
                    negmax = smpool.tile([L, 1], F32, tag="negmax")
                    nc.vector.reduce_max(negmax, sc_sb, axis=AX.X, negate=True)
                    ex_sb = scpool.tile([L, TB], F32, tag="ex")
                    denom = smpool.tile([L, 1], F32, tag="denom")
                    nc.scalar.activation(ex_sb, sc_sb, ACTF.Exp, bias=negmax,
                                         accum_out=denom)
                    rden = smpool.tile([L, 1], F32, tag="rden")
                    nc.vector.reciprocal(rden, denom)

                    # v row-major per 128-row chunk + ex transposed
                    v_sb = vpool.tile([128, RC, LC], F32R)
                    ext_sb = extpool.tile([128, RC, L], F32R)
                    for rc in range(RC):
                        r0 = rc * 128
                        psv = pv.tile([128, LC], F32, tag="psv")
                        for kt in range(LC // 128):
                            nc.tensor.matmul(
                                psv, xt2_sb[:, kt, r0:r0 + 128], wv_sb[:, kt, :],
                                start=(kt == 0), stop=(kt == LC // 128 - 1))
                        evac(v_sb[:, rc, :], psv)
                        pst = pt.tile([128, L], F32, tag="pst")
                        nc.tensor.transpose(pst, ex_sb[:, r0:r0 + 128],
                                            ident_sb[0:L, 0:L])
                        evac(ext_sb[:, rc, :], pst)

                    # out_group[j] = (ex/denom) @ v   -> [L, LC]
                    pso = pog.tile([L, LC], F32, tag="pso")
                    for rc in range(RC):
                        nc.tensor.matmul(pso, ext_sb[:, rc, :], v_sb[:, rc, :],
                                         start=(rc == 0), stop=(rc == RC - 1))
                    nc.vector.tensor_scalar_mul(og_sb[:, j, :], pso, rden)

                nc.sync.dma_start(out=out_og.ap(), in_=og_sb)

            # ---------------- whole-instance branch ----------------
            with (
                tc.tile_pool(name="wg", bufs=1) as wgpool,
                tc.tile_pool(name="wtile", bufs=1) as wtpool,
                tc.tile_pool(name="pw", bufs=2, space="PSUM") as pw,
            ):
                wg0_sb = wgpool.tile([128, IN // 128, 2 * GL], F32R)
                nc.gpsimd.dma_start(out=wg0_sb,
                                  in_=wg0.ap().bitcast(F32R).rearrange(
                                      "(kt p) m -> p kt m", p=128))
                wg1_sb = wgpool.tile([128, 2 * GL // 128, GL], F32R)
                nc.gpsimd.dma_start(out=wg1_sb,
                                  in_=wg1.ap().bitcast(F32R).rearrange(
                                      "(kt p) m -> p kt m", p=128))
                bg0_sb = wgpool.tile([128, 2 * GL // 128], F32)
                nc.gpsimd.dma_start(out=bg0_sb, in_=bg0t.ap())
                bg1_sb = wgpool.tile([128, GL // 128], F32)
                nc.gpsimd.dma_start(out=bg1_sb, in_=bg1t.ap())

                xw_sb = wtpool.tile([128, IN // 128, Wt], F32R)
                nc.gpsimd.dma_start(out=xw_sb,
                                  in_=xwt.ap().bitcast(F32R).rearrange(
                                      "(kt p) t -> p kt t", p=128))
                pfw_sb = wtpool.tile([1, Wt], F32R)
                nc.gpsimd.dma_start(out=pfw_sb, in_=padfw.ap().bitcast(F32R))

                h1w_sb = wtpool.tile([128, 2 * GL // 128, Wt], F32R)
                for mc in range(2 * GL // 128):
                    for (w0, wsz) in wchunks:
                        ps = pw.tile([128, 512], F32, tag="pw")
                        for kt in range(IN // 128):
                            nc.tensor.matmul(
                                ps[:, :wsz],
                                wg0_sb[:, kt, mc * 128:(mc + 1) * 128],
                                xw_sb[:, kt, w0:w0 + wsz],
                                start=(kt == 0), stop=(kt == IN // 128 - 1))
                        evac(h1w_sb[:, mc, w0:w0 + wsz], ps[:, :wsz],
                             bg0_sb[:, mc:mc + 1])

                h2w_sb = wtpool.tile([128, GL // 128, Wt], F32)
                for mc in range(GL // 128):
                    for (w0, wsz) in wchunks:
                        ps = pw.tile([128, 512], F32, tag="pw")
                        for kt in range(2 * GL // 128):
                            nc.tensor.matmul(
                                ps[:, :wsz],
                                wg1_sb[:, kt, mc * 128:(mc + 1) * 128],
                                h1w_sb[:, kt, w0:w0 + wsz],
                                start=(kt == 0),
                                stop=(not whole_aug and kt == 2 * GL // 128 - 1))
                        if whole_aug:
                            nc.tensor.matmul(ps[:, :wsz], negbig_sb[0:1, :],
                                             pfw_sb[0:1, w0:w0 + wsz],
                                             start=False, stop=True)
                        evac(h2w_sb[:, mc, w0:w0 + wsz], ps[:, :wsz],
                             bg1_sb[:, mc:mc + 1])

                wag_sb = wtpool.tile([128, GL // 128, GPC], F32)
                for mc in range(GL // 128):
                    for j in range(GPC):
                        nc.vector.reduce_max(wag_sb[:, mc, j:j + 1],
                                             h2w_sb[:, mc, j * WB:(j + 1) * WB],
                                             axis=AX.X)
                nc.sync.dma_start(out=out_w.ap(), in_=wag_sb)

    nc.compile()
    return nc


def _get_program(key):
    if key not in _prog_cache:
        _prog_cache[key] = _build_program(*key)
    return _prog_cache[key]


def kernel(**inputs):
    x = np.ascontiguousarray(np.asarray(inputs["x"], dtype=np.float32))
    group = np.asarray(inputs["group"]).astype(np.int64)
    itype = np.asarray(inputs["instance_type"]).astype(np.int64)
    Wl0 = np.asarray(inputs["Wl0"], np.float32)
    bl0 = np.asarray(inputs["bl0"], np.float32)
    Wl1 = np.asarray(inputs["Wl1"], np.float32)
    bl1 = np.asarray(inputs["bl1"], np.float32)
    Wg0 = np.asarray(inputs["Wg0"], np.float32)
    bg0 = np.asarray(inputs["bg0"], np.float32)
    Wg1 = np.asarray(inputs["Wg1"], np.float32)
    bg1 = np.asarray(inputs["bg1"], np.float32)
    Wk = np.asarray(inputs["Wk"], np.float32)
    bk = np.asarray(inputs["bk"], np.float32)
    Wv = np.asarray(inputs["Wv"], np.float32)
    bv = np.asarray(inputs["bv"], np.float32)
    latent = np.asarray(inputs["latent"], np.float32)
    Wout = np.asarray(inputs["Wout"], np.float32)
    bout = np.asarray(inputs["bout"], np.float32)

    # ---- host bucketing ----
    is_tile = itype == 1
    is_whole = itype == 0
    tile_idx = [np.where(is_tile & (group == g))[0] for g in range(G)]
    whole_idx = [np.where(is_whole & (group == g))[0] for g in range(G)]
    ng = np.array([len(ix) for ix in tile_idx])
    wg = np.array([len(ix) for ix in whole_idx])
    TB = max(384, _ceil_to(int(ng.max()), 384))
    WB = max(1, int(wg.max()))
    T = GPC * TB
    Wt = GPC * WB
    tile_aug = bool((ng < TB).any())
    whole_aug = bool((wg < WB).any())

    # ---- per-core staged arrays ----
    in_maps = []
    scale = 1.0 / math.sqrt(LC)
    wkl = np.ascontiguousarray((Wk @ latent.T) * scale).astype(np.float32)
    shared = dict(
        wl0=Wl0, wl1=Wl1, wv=np.ascontiguousarray(Wv), wkl=wkl,
        wg0=Wg0, wg1=Wg1,
        bl0t=np.ascontiguousarray(bl0.reshape(-1, 128).T),
        bl1t=np.ascontiguousarray(bl1.reshape(-1, 128).T),
        bg0t=np.ascontiguousarray(bg0.reshape(-1, 128).T),
        bg1t=np.ascontiguousarray(bg1.reshape(-1, 128).T),
    )
    for c in range(N_CORES):
        xtt = np.zeros((IN, T), np.float32)
        xwt = np.zeros((IN, Wt), np.float32)
        padf = np.ones((1, T), np.float32)
        padfw = np.ones((1, Wt), np.float32)
        for j in range(GPC):
            g = c * GPC + j
            ti, wi = tile_idx[g], whole_idx[g]
            xtt[:, j * TB:j * TB + len(ti)] = x[ti].T
            xwt[:, j * WB:j * WB + len(wi)] = x[wi].T
            padf[0, j * TB:j * TB + len(ti)] = 0.0
            padfw[0, j * WB:j * WB + len(wi)] = 0.0
        in_maps.append(dict(xtt=xtt, xwt=xwt, padf=padf, padfw=padfw,
                            negbig=np.full((1, 128), NEGBIG, np.float32),
                            **shared))

    nc = _get_program((TB, WB, tile_aug, whole_aug))
    trace = os.environ.get("KERNEL_TRACE") == "1"
    if trace:
        _install_ntff_shim()
    res = run_bass_kernel_spmd(nc, in_maps, core_ids=list(range(N_CORES)),
                               trace=trace)
    global last_exec_time_ns, last_mean_exec_time_ns
    last_exec_time_ns = res.exec_time_ns
    last_mean_exec_time_ns = res.mean_exec_time_ns

    # ---- host assembly ----
    whole_agg = np.empty((G, GL), np.float32)
    out_group = np.empty((G, L, LC), np.float32)
    for c in range(N_CORES):
        ow = res.results[c]["out_w"]          # [128, GL//128, GPC]
        og = res.results[c]["out_og"]         # [L, GPC, LC]
        wa = ow.transpose(1, 0, 2).reshape(GL, GPC)   # [GL, GPC]
        for j in range(GPC):
            g = c * GPC + j
            whole_agg[g] = wa[:, j]
            if wg[g] == 0:
                whole_agg[g] = -np.inf
            out_group[g] = og[:, j, :] + bv[None, :]
    fused = np.concatenate([whole_agg, out_group.reshape(G, L * LC)], axis=1)
    return (fused @ Wout + bout).astype(np.float32)
